# revision 1
# baseline (speedup 1.0000x reference)
"""AttnBlock (channel attention over 64x64 feature maps) for Trainium2.

Data-parallel over batch: 16 batches sharded 2-per-core across 8 NeuronCores.
Per batch [C=512, N=4096]:
  hn    = GroupNorm(x; 32 groups)              -> folded into per-channel affine A,B
  q/k/v = W @ hn + b                           -> W' = W * A (cols), b' = b + W @ B
  qT,kT computed in [n, o] layout (pixel-major) so scores contract over n on PE
  scores[c,d] = sum_n q[c,n] k[d,n];  attn = softmax(scores * C^-0.5, axis=d)
  out   = Wo @ (attn @ v) + bo;  y = x + out
Big matmuls run in bf16 (hidden weight loads); stats/softmax/residual in fp32.
Batches are software-pipelined: batch b+1's loads and stats run during batch
b's attention phases, so only batch 0 pays the prologue.
"""

import sys

if "/opt/trn_rl_repo" not in sys.path:
    sys.path.insert(0, "/opt/trn_rl_repo")

import numpy as np

C = 512          # channels
N = 4096         # pixels (64*64)
BB = 2           # batches per core
P = 128          # partitions
CB = C // P      # 4 channel blocks
NT = N // P      # 32 pixel tiles of 128 (phase B)
NSL = 512        # pixel slice width (phase EF)
NS = N // NSL    # 8 pixel slices
GROUPS = 32
EPS = 1e-6
SCALE = float(C) ** -0.5

_NC_CACHE = {}
LAST_RESULT = None


def _build_nc():
    import concourse.bacc as bacc
    import concourse.tile as tile
    from concourse import mybir
    from concourse.bass import ts

    F32 = mybir.dt.float32
    F32R = mybir.dt.float32r
    BF16 = mybir.dt.bfloat16
    AX = mybir.AxisListType
    AF = mybir.ActivationFunctionType
    OP = mybir.AluOpType

    nc = bacc.Bacc(None, target_bir_lowering=False, num_swdge_queues=4)

    xs_d = nc.dram_tensor("xs", [BB, C, N], F32, kind="ExternalInput")
    xsb_d = nc.dram_tensor("xsb", [BB, C, N], BF16, kind="ExternalInput")
    wqt_d = nc.dram_tensor("wqtb", [C, C], BF16, kind="ExternalInput")
    wkt_d = nc.dram_tensor("wktb", [C, C], BF16, kind="ExternalInput")
    wvt_d = nc.dram_tensor("wvtb", [C, C], BF16, kind="ExternalInput")
    wot_d = nc.dram_tensor("wotb", [C, C], BF16, kind="ExternalInput")
    bq_d = nc.dram_tensor("bq", [C], F32, kind="ExternalInput")
    bk_d = nc.dram_tensor("bk", [C], F32, kind="ExternalInput")
    bv_d = nc.dram_tensor("bv", [C], F32, kind="ExternalInput")
    bo_d = nc.dram_tensor("bo", [C], F32, kind="ExternalInput")
    gamma_d = nc.dram_tensor("gamma", [C], F32, kind="ExternalInput")
    beta_d = nc.dram_tensor("beta", [C], F32, kind="ExternalInput")
    gfwd_d = nc.dram_tensor("gfwd", [P, CB, GROUPS], F32, kind="ExternalInput")
    gbwd_d = nc.dram_tensor("gbwd", [GROUPS, CB, P], F32, kind="ExternalInput")
    y_d = nc.dram_tensor("y", [BB, C, N], F32, kind="ExternalOutput")

    WKEYS = ("q", "k", "v")

    with tile.TileContext(nc) as tc:
        with (
            tc.tile_pool(name="singles", bufs=1) as sg,
            tc.tile_pool(name="sbp", bufs=1) as sbp,
            tc.tile_pool(name="psp", bufs=1, space="PSUM") as psp,
            tc.tile_pool(name="drp", bufs=1, space="DRAM") as drp,
        ):
            xview = [xs_d[b].rearrange("(cb p) n -> p cb n", p=P) for b in range(BB)]
            xbview = [xsb_d[b].rearrange("(cb p) n -> p cb n", p=P) for b in range(BB)]
            yview = [y_d[b].rearrange("(ob p) n -> p ob n", p=P) for b in range(BB)]
            wt_dram = {"q": wqt_d, "k": wkt_d, "v": wvt_d}
            bias_dram = {}
            st = [dict() for _ in range(BB)]  # per-batch tile state

            def emit_load(b):
                """x (bf16 cast) + raw weight chunks (bf16 cast). DMA only."""
                s = st[b]
                xbf = sbp.tile([P, CB, N], BF16, tag="xbf", bufs=2, name=f"xbf{b}")
                s["xbf"] = xbf
                wall = sbp.tile(
                    [P, 3, CB, C], BF16, tag="wall", bufs=1, name=f"wall{b}"
                )
                s["wall"] = wall
                for cb in range(CB):
                    nc.sync.dma_start(xbf[:, cb, :], xbview[b][:, cb, :])
                for wi, w in enumerate(WKEYS):
                    for cb in range(CB):
                        nc.sync.dma_start(
                            wall[:, wi, cb, :], wt_dram[w][ts(cb, P), :]
                        )

            def emit_stats(b, split=False):
                """Per-channel [mean, E[x^2]] -> t. DVE bn_stats; batch 0
                spreads blocks across DVE/ACT/GpSimd (all idle in prologue)."""
                s = st[b]
                xbf = s["xbf"]
                t = sbp.tile([P, CB, 2], F32, tag="t", bufs=2, name=f"t{b}")
                act_cbs = (0, 1) if split else ()
                gps_cbs = ()
                bn_cbs = [cb for cb in range(CB)
                          if cb not in act_cbs and cb not in gps_cbs]
                stats = sbp.tile(
                    [P, CB, 8, 6], F32, tag="stats", bufs=2, name=f"st{b}"
                )
                mv = sbp.tile([P, CB, 2], F32, tag="mv", bufs=2, name=f"mv{b}")
                for cb in act_cbs:
                    sq = sbp.tile([P, N], F32, tag="vfull", bufs=1,
                                  name=f"sq{b}{cb}")
                    s1 = sbp.tile([P, 1], F32, tag="s1", bufs=2, name=f"s1{b}{cb}")
                    s2 = sbp.tile([P, 1], F32, tag="s2", bufs=2, name=f"s2{b}{cb}")
                    nc.scalar.activation(
                        sq, xbf[:, cb, :], AF.Copy, accum_out=s1
                    )
                    nc.scalar.activation(
                        sq, xbf[:, cb, :], AF.Square, accum_out=s2
                    )
                    nc.vector.tensor_scalar_mul(t[:, cb, 0:1], s1, 1.0 / N)
                    nc.vector.tensor_scalar_mul(t[:, cb, 1:2], s2, 1.0 / N)
                for cb in gps_cbs:
                    for j in range(8):
                        nc.gpsimd.bn_stats(
                            stats[:, cb, j, :], xbf[:, cb, ts(j, 512)]
                        )
                    nc.gpsimd.bn_aggr(mv[:, cb, :], stats[:, cb, :, :])
                for cb in bn_cbs:
                    for j in range(8):
                        nc.vector.bn_stats(
                            stats[:, cb, j, :], xbf[:, cb, ts(j, 512)]
                        )
                    nc.vector.bn_aggr(mv[:, cb, :], stats[:, cb, :, :])
                for cb in list(gps_cbs) + bn_cbs:
                    nc.vector.tensor_mul(
                        t[:, cb, 1:2], mv[:, cb, 0:1], mv[:, cb, 0:1]
                    )
                    nc.vector.tensor_add(
                        t[:, cb, 1:2], t[:, cb, 1:2], mv[:, cb, 1:2]
                    )
                    nc.vector.tensor_copy(t[:, cb, 0:1], mv[:, cb, 0:1])
                s["t"] = t

            def emit_a2(b):
                """Group aggregation, A/B affine, bias folding, weight scaling."""
                s = st[b]
                t, wall = s["t"], s["wall"]
                pg = psp.tile([GROUPS, 2], F32, tag="work", bufs=4, name=f"pg{b}")
                for cb in range(CB):
                    nc.tensor.matmul(
                        pg, gfwd[:, cb, :], t[:, cb, :],
                        start=(cb == 0), stop=(cb == CB - 1),
                    )
                gs = sbp.tile([GROUPS, 2], F32, tag="gs", bufs=2, name=f"gs{b}")
                pgs = sbp.tile([GROUPS, 2], F32, tag="pgs", bufs=2, name=f"pgs{b}")
                nc.vector.tensor_copy(pgs, pg)
                vtmp = sbp.tile([GROUPS, 1], F32, tag="vtmp", bufs=2, name=f"vt{b}")
                nc.vector.tensor_mul(vtmp, pgs[:, 0:1], pgs[:, 0:1])
                nc.vector.tensor_tensor(vtmp, pgs[:, 1:2], vtmp, op=OP.subtract)
                nc.vector.tensor_copy(gs[:, 0:1], pgs[:, 0:1])
                nc.scalar.activation(gs[:, 1:2], vtmp, AF.Sqrt, bias=eps_g)
                nc.vector.reciprocal(gs[:, 1:2], gs[:, 1:2])

                cst = sbp.tile([P, CB, 2], F32, tag="cst", bufs=2, name=f"cs{b}")
                for cb in range(CB):
                    pc = psp.tile([P, 2], F32, tag="work", bufs=4, name=f"pc{b}_{cb}")
                    nc.tensor.matmul(pc, gbwd[:, cb, :], gs, start=True, stop=True)
                    nc.vector.tensor_copy(cst[:, cb, :], pc)

                A_ = sbp.tile([P, CB], F32, tag="A_", bufs=2, name=f"A{b}")
                Bb = sbp.tile([P, CB], BF16, tag="Bb", bufs=2, name=f"B{b}")
                tmpB = sbp.tile([P, CB], F32, tag="tmpB", bufs=2, name=f"tB{b}")
                nc.vector.tensor_mul(A_, cst[:, :, 1], gam)
                nc.vector.tensor_mul(tmpB, cst[:, :, 0], A_)
                nc.vector.tensor_tensor(Bb, bet, tmpB, op=OP.subtract)

                wq_p = sbp.tile([P, CB, C], BF16, tag="wq_p", bufs=2, name=f"wq{b}")
                wk_p = sbp.tile([P, CB, C], BF16, tag="wk_p", bufs=2, name=f"wk{b}")
                wv_p = sbp.tile([P, CB, C], BF16, tag="wv_p", bufs=2, name=f"wv{b}")
                s["wq_p"], s["wk_p"], s["wv_p"] = wq_p, wk_p, wv_p
                wsc_map = {"q": wq_p, "k": wk_p, "v": wv_p}
                for wi, w in enumerate(WKEYS):
                    wsc = wsc_map[w]
                    pb = psp.tile([1, C], F32, tag="work", bufs=4, name=f"pb{b}{w}")
                    for cb in range(CB):
                        nc.vector.tensor_scalar_mul(
                            wsc[:, cb, :], wall[:, wi, cb, :], A_[:, cb : cb + 1]
                        )
                        nc.tensor.matmul(
                            pb, Bb[:, cb : cb + 1], wall[:, wi, cb, :],
                            start=(cb == 0), stop=(cb == CB - 1),
                        )
                    if w in ("q", "k"):
                        bfull = sbp.tile([1, C], BF16, tag=f"bf_{w}", bufs=2,
                                         name=f"bf{b}{w}")
                        nc.vector.tensor_add(bfull, pb, bias_dram[w])
                        # broadcast to all partitions via rank-1 PE matmul
                        pbc = psp.tile([P, C], F32, tag="work", bufs=4,
                                       name=f"pbc{b}{w}")
                        nc.tensor.matmul(pbc, ones1, bfull, start=True, stop=True)
                        dst = sbp.tile(
                            [P, C], F32, tag=f"b{w}b", bufs=1, name=f"b{w}b{b}"
                        )
                        nc.vector.tensor_copy(dst, pbc)
                        s[f"b{w}b"] = dst
                    else:
                        bfull = sbp.tile([1, C], F32, tag=f"bf_{w}", bufs=2,
                                         name=f"bf{b}{w}")
                        nc.vector.tensor_add(bfull, pb, bias_dram[w])
                        scr = drp.tile([C], F32, name=f"scr{b}{w}")
                        nc.sync.dma_start(scr.rearrange("(a c) -> a c", a=1), bfull)
                        bvb = sbp.tile([P, CB], F32, tag="bvb", bufs=2,
                                       name=f"bvb{b}")
                        nc.sync.dma_start(
                            bvb, scr.rearrange("(cb p) -> p cb", p=P)
                        )
                        s["bvb"] = bvb

            def emit_b(b):
                """qT/kT pixel-tiles + score accumulation."""
                s = st[b]
                xbf, wq_p, wk_p = s["xbf"], s["wq_p"], s["wk_p"]
                bqb, bkb = s["bqb"], s["bkb"]
                scores = [
                    psp.tile([P, C], F32, tag="scores", bufs=4, name=f"sc{b}_{cb}")
                    for cb in range(CB)
                ]
                s["scores"] = scores
                for i in range(NT):
                    psq = psp.tile([P, C], F32, tag="work", bufs=4,
                                   name=f"psq{b}_{i}")
                    for cb in range(CB):
                        nc.tensor.matmul(
                            psq, xbf[:, cb, ts(i, P)], wq_p[:, cb, :],
                            start=(cb == 0), stop=(cb == CB - 1),
                        )
                    qt = sbp.tile([P, C], BF16, tag="qt", bufs=3, name=f"qt{b}_{i}")
                    nc.vector.tensor_add(qt, psq, bqb)

                    psk = psp.tile([P, C], F32, tag="work", bufs=4,
                                   name=f"psk{b}_{i}")
                    for cb in range(CB):
                        nc.tensor.matmul(
                            psk, xbf[:, cb, ts(i, P)], wk_p[:, cb, :],
                            start=(cb == 0), stop=(cb == CB - 1),
                        )
                    kt = sbp.tile([P, C], BF16, tag="kt", bufs=3, name=f"kt{b}_{i}")
                    nc.vector.tensor_add(kt, psk, bkb)

                    for cb in range(CB):
                        nc.tensor.matmul(
                            scores[cb], qt[:, ts(cb, P)], kt,
                            start=(i == 0), stop=(i == NT - 1),
                        )

            def emit_softmax(b):
                s = st[b]
                scores = s["scores"]
                e_sb = sbp.tile([P, CB, C], BF16, tag="e", bufs=1, name=f"e{b}")
                rinv = sbp.tile([P, CB], F32, tag="rinv", bufs=1, name=f"ri{b}")
                s["e"], s["rinv"] = e_sb, rinv
                for cb in range(CB):
                    rs = sbp.tile([P, 1], F32, tag="rs", bufs=2, name=f"rs{b}{cb}")
                    nc.scalar.activation(
                        e_sb[:, cb, :], scores[cb], AF.Exp,
                        bias=0.0, scale=SCALE, accum_out=rs,
                    )
                    nc.vector.reciprocal(rinv[:, cb : cb + 1], rs)

            def emit_t(b):
                """Transpose e -> eT via DMA transpose (bf16, 128x128 blocks)."""
                s = st[b]
                e_sb = s["e"]
                eT = sbp.tile([P, CB, C], BF16, tag="eT", bufs=1, name=f"eT{b}")
                s["eT"] = eT
                for cb in range(CB):
                    for db in range(CB):
                        nc.sync.dma_start(
                            eT[:, db, ts(cb, P)],
                            e_sb[:, cb, ts(db, P)],
                            transpose=True,
                        )

            def emit_v(b):
                """v projection — independent of softmax, keeps PE busy."""
                s = st[b]
                xbf, wv_p, bvb = s["xbf"], s["wv_p"], s["bvb"]
                vfull = sbp.tile([P, CB, N], BF16, tag="vfull", bufs=1,
                                 name=f"v{b}")
                s["vfull"] = vfull
                for nsl in range(NS):
                    for ob in range(CB):
                        pv = psp.tile([P, NSL], F32, tag="work", bufs=4,
                                      name=f"pv{b}{nsl}{ob}")
                        for cb in range(CB):
                            nc.tensor.matmul(
                                pv, wv_p[:, cb, ts(ob, P)],
                                xbf[:, cb, ts(nsl, NSL)],
                                start=(cb == 0), stop=(cb == CB - 1),
                            )
                        if (nsl * CB + ob) % 4 == 3:
                            nc.vector.tensor_scalar_add(
                                vfull[:, ob, ts(nsl, NSL)], pv,
                                bvb[:, ob : ob + 1],
                            )
                        else:
                            nc.scalar.add(
                                vfull[:, ob, ts(nsl, NSL)], pv,
                                bvb[:, ob : ob + 1],
                            )

            def emit_ef(b, early_free=False):
                s = st[b]
                eT, vfull, rinv = s["eT"], s["vfull"], s["rinv"]
                for nsl in range(NS):
                    pf_tag = "work" if (early_free and nsl >= NS - 1) else "scores"
                    xsl = sbp.tile([P, CB, NSL], F32, tag="xsl", bufs=2,
                                   name=f"xs{b}_{nsl}")
                    for cb in range(CB):
                        nc.gpsimd.dma_start(
                            xsl[:, cb, :], xview[b][:, cb, ts(nsl, NSL)]
                        )
                    ao = sbp.tile([P, CB, NSL], BF16, tag="ao", bufs=2,
                                  name=f"ao{b}_{nsl}")
                    for cb in range(CB):
                        pa = psp.tile([P, NSL], F32, tag="work", bufs=4,
                                      name=f"pa{b}{nsl}{cb}")
                        for db in range(CB):
                            nc.tensor.matmul(
                                pa, eT[:, db, ts(cb, P)],
                                vfull[:, db, ts(nsl, NSL)],
                                start=(db == 0), stop=(db == CB - 1),
                            )
                        nc.scalar.mul(ao[:, cb, :], pa, rinv[:, cb : cb + 1])

                    for ob in range(CB):
                        pf = psp.tile([P, NSL], F32, tag=pf_tag, bufs=4,
                                      name=f"pf{b}{nsl}{ob}")
                        for cb in range(CB):
                            nc.tensor.matmul(
                                pf, wot[:, cb, ts(ob, P)], ao[:, cb, :],
                                start=(cb == 0), stop=(cb == CB - 1),
                            )
                        yt = sbp.tile([P, NSL], F32, tag="yt", bufs=3,
                                      name=f"yt{b}{nsl}{ob}")
                        nc.vector.scalar_tensor_tensor(
                            yt, pf, bob[:, ob : ob + 1], xsl[:, ob, :],
                            op0=OP.add, op1=OP.add,
                        )
                        nc.sync.dma_start(yview[b][:, ob, ts(nsl, NSL)], yt)

            # ---- software-pipelined emission across the two batches ----
            emit_load(0)
            # HAM warm-up: keep TensorE busy through the prologue so phase B
            # starts at full clock. Burst flips K to 8/8; pokes tied to the
            # arriving x chunks stop the MID-window re-throttle. The dummy
            # accumulator drains to DRAM so the chain is not dead code.
            zsb = sg.tile([P, NSL], BF16, name="zsb")
            nc.gpsimd.memset(zsb, 0.0)
            pdum = psp.tile([P, NSL], F32, tag="work", bufs=4, name="pdum")
            for i in range(24):
                nc.tensor.matmul(
                    pdum, zsb[:, :P], zsb, start=(i == 0), stop=False
                )
            for cb in range(CB):
                nc.tensor.matmul(
                    pdum, st[0]["xbf"][:, cb, ts(0, P)], zsb,
                    start=False, stop=(cb == CB - 1),
                )
            dsb = sg.tile([1, 1], F32, name="dsb")
            nc.vector.tensor_copy(dsb, pdum[0:1, 0:1])
            dscr = drp.tile([1], F32, name="dscr")
            nc.sync.dma_start(dscr.rearrange("(a c) -> a c", a=1), dsb)
            # ---- constants, loaded once ----
            gfwd = sg.tile([P, CB, GROUPS], F32)
            nc.sync.dma_start(gfwd, gfwd_d[:])
            gbwd = sg.tile([GROUPS, CB, P], F32)
            nc.sync.dma_start(gbwd, gbwd_d[:])
            wot = sg.tile([P, CB, C], BF16)
            nc.sync.dma_start(wot, wot_d[:].rearrange("(cb p) o -> p cb o", p=P))
            gam = sg.tile([P, CB], F32)
            nc.sync.dma_start(gam, gamma_d[:].rearrange("(cb p) -> p cb", p=P))
            bet = sg.tile([P, CB], F32)
            nc.sync.dma_start(bet, beta_d[:].rearrange("(cb p) -> p cb", p=P))
            bob = sg.tile([P, CB], F32)
            nc.sync.dma_start(bob, bo_d[:].rearrange("(cb p) -> p cb", p=P))
            bqv = sg.tile([1, C], F32)
            nc.sync.dma_start(bqv, bq_d[:].rearrange("(a c) -> a c", a=1))
            bkv = sg.tile([1, C], F32)
            nc.sync.dma_start(bkv, bk_d[:].rearrange("(a c) -> a c", a=1))
            bvv = sg.tile([1, C], F32)
            nc.sync.dma_start(bvv, bv_d[:].rearrange("(a c) -> a c", a=1))
            eps_g = sg.tile([GROUPS, 1], F32)
            nc.vector.memset(eps_g, EPS)
            ones1 = sg.tile([1, P], BF16)
            nc.vector.memset(ones1, 1.0)
            bias_dram["q"], bias_dram["k"], bias_dram["v"] = bqv, bkv, bvv

            emit_stats(0, split=True)
            emit_a2(0)
            for b in range(BB):
                emit_b(b)
                if b + 1 < BB:
                    emit_load(b + 1)
                emit_softmax(b)
                emit_t(b)
                if b + 1 < BB:
                    emit_stats(b + 1)
                emit_v(b)
                if b + 1 < BB:
                    emit_a2(b + 1)
                emit_ef(b, early_free=(b + 1 < BB))

    nc.finalize()
    return nc


def _get_nc():
    if "nc" not in _NC_CACHE:
        _NC_CACHE["nc"] = _build_nc()
    return _NC_CACHE["nc"]


def _make_consts():
    gfwd = np.zeros((P, CB, GROUPS), np.float32)
    gbwd = np.zeros((GROUPS, CB, P), np.float32)
    for cb in range(CB):
        for p in range(P):
            g = (cb * P + p) // 16
            gfwd[p, cb, g] = 1.0 / 16.0
            gbwd[g, cb, p] = 1.0
    ident = np.eye(P, dtype=np.float32)
    return gfwd, gbwd, ident


def kernel(x, gamma, beta, Wq, bq, Wk, bk, Wv, bv, Wo, bo):
    global LAST_RESULT
    from concourse.bass_utils import run_bass_kernel_spmd

    import ml_dtypes

    BF = ml_dtypes.bfloat16
    x = np.ascontiguousarray(np.asarray(x, np.float32)).reshape(16, C, N)
    xb16 = np.ascontiguousarray(x.astype(BF))
    gfwd, gbwd, ident = _make_consts()
    shared = {
        "wqtb": np.ascontiguousarray(np.asarray(Wq, np.float32).T.astype(BF)),
        "wktb": np.ascontiguousarray(np.asarray(Wk, np.float32).T.astype(BF)),
        "wvtb": np.ascontiguousarray(np.asarray(Wv, np.float32).T.astype(BF)),
        "wotb": np.ascontiguousarray(np.asarray(Wo, np.float32).T.astype(BF)),
        "bq": np.ascontiguousarray(np.asarray(bq, np.float32)),
        "bk": np.ascontiguousarray(np.asarray(bk, np.float32)),
        "bv": np.ascontiguousarray(np.asarray(bv, np.float32)),
        "bo": np.ascontiguousarray(np.asarray(bo, np.float32)),
        "gamma": np.ascontiguousarray(np.asarray(gamma, np.float32)),
        "beta": np.ascontiguousarray(np.asarray(beta, np.float32)),
        "gfwd": gfwd,
        "gbwd": gbwd,
    }
    in_maps = [
        dict(
            shared,
            xs=np.ascontiguousarray(x[BB * i : BB * (i + 1)]),
            xsb=np.ascontiguousarray(xb16[BB * i : BB * (i + 1)]),
        )
        for i in range(8)
    ]
    nc = _get_nc()
    import os

    trace = os.environ.get("KERNEL_TRACE") == "1"
    res = run_bass_kernel_spmd(nc, in_maps, core_ids=list(range(8)), trace=trace)
    LAST_RESULT = res
    y = np.concatenate([r["y"] for r in res.results], axis=0)
    return y.reshape(16, C, 64, 64)



# revision 5
# speedup vs baseline: 2.2300x; 2.2300x over previous
"""AttnBlock (channel attention over 64x64 maps) for Trainium2 — Gram form.

Data-parallel over batch: 16 batches, 2 per core across 8 NeuronCores.
Per batch [C=512, N=4096], with hn = A.x + B (GroupNorm folded to affine):
  scores = q k^T = (WqA) XX (WkA)^T + u bk'^T + bq' (w + N bk')^T,  XX = x x^T
  attn   = softmax(scores * C^-0.5)  (no max-sub; e kept in bf16)
  out    = Wo attn v = FWt^T (A.x) + (F bv')1^T, F = Wo D_rinv E, FWt = Wv^T F^T
  y      = x + out + bo
So the N-wide work is only XX (1 unit) and the final FWt^T @ xA (1 unit);
q/k/v projections and the output projection collapse into C x C x C matmuls
(0.125 unit each). ~2.5 units/batch vs 6 for the direct form.
Matmuls run fp16 (weights/x/XX/M1/Ft/FWt) or bf16 (e, rinv-scaled Wo — range),
stats/softmax/residual fp32.
"""

import sys

if "/opt/trn_rl_repo" not in sys.path:
    sys.path.insert(0, "/opt/trn_rl_repo")

import numpy as np

C = 512          # channels
N = 4096         # pixels (64*64)
BB = 2           # batches per core
P = 128          # partitions
CB = C // P      # 4 channel blocks
NT = N // P      # 32 pixel tiles of 128 (Gram phase)
NTH = NT // 2    # half split of pixel tiles (two xt tiles => earlier prefetch)
NSL = 512        # pixel slice width (epilogue)
NS = N // NSL    # 8 pixel slices
GROUPS = 32
EPS = 1e-6
SCALE = float(C) ** -0.5

_NC_CACHE = {}
LAST_RESULT = None


def _build_nc():
    import concourse.bacc as bacc
    import concourse.tile as tile
    from concourse import mybir
    from concourse.bass import ts

    F32 = mybir.dt.float32
    BF16 = mybir.dt.bfloat16
    FP16 = mybir.dt.float16
    AF = mybir.ActivationFunctionType
    OP = mybir.AluOpType

    nc = bacc.Bacc(None, target_bir_lowering=False, num_swdge_queues=4)

    xs_d = nc.dram_tensor("xs", [BB, C, N], BF16, kind="ExternalInput")
    xt_d = nc.dram_tensor("xt", [BB, N, C], FP16, kind="ExternalInput")
    wqt_d = nc.dram_tensor("wqt", [C, C], FP16, kind="ExternalInput")
    wkt_d = nc.dram_tensor("wkt", [C, C], FP16, kind="ExternalInput")
    wvt_d = nc.dram_tensor("wvt", [C, C], FP16, kind="ExternalInput")
    wvnt_d = nc.dram_tensor("wvnt", [C, C], FP16, kind="ExternalInput")
    wot_d = nc.dram_tensor("wot", [C, C], FP16, kind="ExternalInput")
    bq_d = nc.dram_tensor("bq", [C], F32, kind="ExternalInput")
    bk_d = nc.dram_tensor("bk", [C], F32, kind="ExternalInput")
    bv_d = nc.dram_tensor("bv", [C], F32, kind="ExternalInput")
    bo_d = nc.dram_tensor("bo", [C], F32, kind="ExternalInput")
    gamma_d = nc.dram_tensor("gamma", [C], F32, kind="ExternalInput")
    beta_d = nc.dram_tensor("beta", [C], F32, kind="ExternalInput")
    gfwd_d = nc.dram_tensor("gfwd", [P, CB, GROUPS], F32, kind="ExternalInput")
    gbwd_d = nc.dram_tensor("gbwd", [GROUPS, CB, P], F32, kind="ExternalInput")
    y_d = nc.dram_tensor("y", [BB, C, N], F32, kind="ExternalOutput")

    WKEYS = ("q", "k", "v")

    with tile.TileContext(nc) as tc:
        with (
            tc.tile_pool(name="singles", bufs=1) as sg,
            tc.tile_pool(name="sbp", bufs=1) as sbp,
            tc.tile_pool(name="psp", bufs=1, space="PSUM") as psp,
            tc.tile_pool(name="drp", bufs=1, space="DRAM") as drp,
        ):
            xview = [xs_d[b].rearrange("(cb p) n -> p cb n", p=P) for b in range(BB)]
            xtview = [xt_d[b].rearrange("(nt p) c -> p nt c", p=P) for b in range(BB)]
            yview = [y_d[b].rearrange("(ob p) n -> p ob n", p=P) for b in range(BB)]
            wt_dram = {"q": wqt_d, "k": wkt_d, "v": wvt_d}
            bias_dram = {}
            st = [dict() for _ in range(BB)]  # per-batch tile state

            def emit_load(b):
                """x (bf16) + xT halves (fp16). DMA only."""
                s = st[b]
                xsb = sbp.tile([P, CB, N], BF16, tag="xsb", bufs=2, name=f"xsb{b}")
                s["xsb"] = xsb
                for cb in range(CB):
                    nc.sync.dma_start(xsb[:, cb, :], xview[b][:, cb, :])
                xta = sbp.tile([P, NTH, C], FP16, tag="xta", bufs=1, name=f"xta{b}")
                xtb = sbp.tile([P, NTH, C], FP16, tag="xtb", bufs=1, name=f"xtb{b}")
                s["xta"], s["xtb"] = xta, xtb
                for h in range(4):
                    nc.sync.dma_start(
                        xta[:, ts(h, 4), :], xtview[b][:, 4 * h : 4 * h + 4, :]
                    )
                for h in range(4):
                    nc.sync.dma_start(
                        xtb[:, ts(h, 4), :],
                        xtview[b][:, NTH + 4 * h : NTH + 4 * h + 4, :],
                    )

            def emit_stats(b):
                """Per-channel [mean, E[x^2]] -> t via DVE bn_stats."""
                s = st[b]
                xsb = s["xsb"]
                t = sbp.tile([P, CB, 2], F32, tag="t", bufs=2, name=f"t{b}")
                stats = sbp.tile(
                    [P, CB, 8, 6], F32, tag="stats", bufs=2, name=f"st{b}"
                )
                mv = sbp.tile([P, CB, 2], F32, tag="mv", bufs=2, name=f"mv{b}")
                for cb in range(CB):
                    for j in range(8):
                        nc.vector.bn_stats(
                            stats[:, cb, j, :], xsb[:, cb, ts(j, 512)]
                        )
                    nc.vector.bn_aggr(mv[:, cb, :], stats[:, cb, :, :])
                for cb in range(CB):
                    nc.vector.tensor_mul(
                        t[:, cb, 1:2], mv[:, cb, 0:1], mv[:, cb, 0:1]
                    )
                    nc.vector.tensor_add(
                        t[:, cb, 1:2], t[:, cb, 1:2], mv[:, cb, 1:2]
                    )
                    nc.vector.tensor_copy(t[:, cb, 0:1], mv[:, cb, 0:1])
                s["t"] = t

            def emit_a2(b):
                """Group aggregation -> A/B affine; scaled weights; bias rows;
                rank-1 score terms; A-scaled x."""
                s = st[b]
                t = s["t"]
                pg = psp.tile([GROUPS, 2], F32, tag="work", bufs=4, name=f"pg{b}")
                for cb in range(CB):
                    nc.tensor.matmul(
                        pg, gfwd[:, cb, :], t[:, cb, :],
                        start=(cb == 0), stop=(cb == CB - 1),
                    )
                gs = sbp.tile([GROUPS, 2], F32, tag="gs", bufs=2, name=f"gs{b}")
                pgs = sbp.tile([GROUPS, 2], F32, tag="pgs", bufs=2, name=f"pgs{b}")
                nc.vector.tensor_copy(pgs, pg)
                vtmp = sbp.tile([GROUPS, 1], F32, tag="vtmp", bufs=2, name=f"vt{b}")
                nc.vector.tensor_mul(vtmp, pgs[:, 0:1], pgs[:, 0:1])
                nc.vector.tensor_tensor(vtmp, pgs[:, 1:2], vtmp, op=OP.subtract)
                nc.vector.tensor_copy(gs[:, 0:1], pgs[:, 0:1])
                nc.scalar.activation(gs[:, 1:2], vtmp, AF.Sqrt, bias=eps_g)
                nc.vector.reciprocal(gs[:, 1:2], gs[:, 1:2])

                cst = sbp.tile([P, CB, 2], F32, tag="cst", bufs=2, name=f"cs{b}")
                for cb in range(CB):
                    pc = psp.tile([P, 2], F32, tag="work", bufs=4, name=f"pc{b}_{cb}")
                    nc.tensor.matmul(pc, gbwd[:, cb, :], gs, start=True, stop=True)
                    nc.vector.tensor_copy(cst[:, cb, :], pc)

                A_ = sbp.tile([P, CB], F32, tag="A_", bufs=2, name=f"A{b}")
                Bb = sbp.tile([P, CB], FP16, tag="Bb", bufs=2, name=f"B{b}")
                tmpB = sbp.tile([P, CB], F32, tag="tmpB", bufs=2, name=f"tB{b}")
                nc.vector.tensor_mul(A_, cst[:, :, 1], gam)
                nc.vector.tensor_mul(tmpB, cst[:, :, 0], A_)
                nc.vector.tensor_tensor(Bb, bet, tmpB, op=OP.subtract)

                wq_p = sbp.tile([P, CB, C], FP16, tag="wq_p", bufs=1, name=f"wq{b}")
                wk_p = sbp.tile([P, CB, C], FP16, tag="wk_p", bufs=1, name=f"wk{b}")
                s["wq_p"], s["wk_p"] = wq_p, wk_p
                for wi, wsc in ((0, wq_p), (1, wk_p)):
                    for cb in range(CB):
                        nc.vector.tensor_scalar_mul(
                            wsc[:, cb, :], wall[:, wi, cb, :], A_[:, cb : cb + 1]
                        )
                # folded bias rows b'_w = W @ B + b_w  (bf16 rows for rank-1 MMs)
                rows = {}
                for wi, w in enumerate(WKEYS):
                    pb = psp.tile([1, C], F32, tag="work", bufs=4, name=f"pb{b}{w}")
                    for cb in range(CB):
                        nc.tensor.matmul(
                            pb, Bb[:, cb : cb + 1], wall[:, wi, cb, :],
                            start=(cb == 0), stop=(cb == CB - 1),
                        )
                    if w in ("q", "k"):
                        bfull = sbp.tile([1, C], BF16, tag=f"bf_{w}", bufs=2,
                                         name=f"bf{b}{w}")
                        nc.vector.tensor_add(bfull, pb, bias_dram[w])
                        rows[w] = bfull
                    else:
                        bfull = sbp.tile([1, C], F32, tag="bf_v", bufs=2,
                                         name=f"bf{b}{w}")
                        nc.vector.tensor_add(bfull, pb, bias_dram[w])
                        scr = drp.tile([C], F32, name=f"scr{b}{w}")
                        nc.sync.dma_start(scr.rearrange("(a c) -> a c", a=1), bfull)
                        bvb = sbp.tile([P, CB], F32, tag="bvb", bufs=2,
                                       name=f"bvb{b}")
                        nc.sync.dma_start(
                            bvb, scr.rearrange("(cb p) -> p cb", p=P)
                        )
                        bvbh = sbp.tile([P, CB], FP16, tag="bvbh", bufs=2,
                                        name=f"bvbh{b}")
                        nc.vector.tensor_copy(bvbh, bvb)
                        s["bvbh"] = bvbh
                s["bq_row"], s["bk_row"] = rows["q"], rows["k"]
                # rank-1 terms: u = WqA sx, w2 = WkA sx + N bk'
                sxc = sbp.tile([P, CB], FP16, tag="sxc", bufs=2, name=f"sx{b}")
                nc.vector.tensor_scalar_mul(sxc, t[:, :, 0], float(N))
                urow = sbp.tile([1, C], BF16, tag="urow", bufs=2, name=f"u{b}")
                wrow = sbp.tile([1, C], BF16, tag="wrow", bufs=2, name=f"w{b}")
                for wsc, dst in ((wq_p, urow), (wk_p, wrow)):
                    pu = psp.tile([1, C], F32, tag="work", bufs=4,
                                  name=f"pu{b}{dst.name}")
                    for cb in range(CB):
                        nc.tensor.matmul(
                            pu, sxc[:, cb : cb + 1], wsc[:, cb, :],
                            start=(cb == 0), stop=(cb == CB - 1),
                        )
                    nc.vector.tensor_copy(dst, pu)
                w2row = sbp.tile([1, C], BF16, tag="w2row", bufs=2, name=f"w2{b}")
                nc.vector.scalar_tensor_tensor(
                    w2row, rows["k"], float(N), wrow, op0=OP.mult, op1=OP.add
                )
                s["urow"], s["w2row"] = urow, w2row
                s["A_"] = A_

            def emit_xx(b):
                """Gram matrix XX = x x^T from fp16 xT tiles."""
                s = st[b]
                xta, xtb = s["xta"], s["xtb"]
                xxps = [
                    psp.tile([P, C], F32, tag="scores", bufs=4, name=f"xx{b}_{i}")
                    for i in range(CB)
                ]
                for nt in range(NT):
                    src = xta if nt < NTH else xtb
                    idx = nt % NTH
                    for i in range(CB):
                        nc.tensor.matmul(
                            xxps[i], src[:, idx, ts(i, P)], src[:, idx, :],
                            start=(nt == 0), stop=(nt == NT - 1),
                        )
                xxsb = sbp.tile([P, CB, C], FP16, tag="xxsb", bufs=1, name=f"xxs{b}")
                s["xxsb"] = xxsb
                for i in range(CB):
                    nc.vector.tensor_copy(xxsb[:, i, :], xxps[i])

            def emit_m1(b):
                """M1 = XX @ (WqA)^T  [e, c]."""
                s = st[b]
                xxsb, wq_p = s["xxsb"], s["wq_p"]
                m1sb = sbp.tile([P, CB, C], FP16, tag="m1sb", bufs=1, name=f"m1{b}")
                s["m1sb"] = m1sb
                for eb in range(CB):
                    m1ps = psp.tile([P, C], F32, tag="work", bufs=4,
                                    name=f"m1p{b}_{eb}")
                    for fb in range(CB):
                        nc.tensor.matmul(
                            m1ps, xxsb[:, fb, ts(eb, P)], wq_p[:, fb, :],
                            start=(fb == 0), stop=(fb == CB - 1),
                        )
                    nc.vector.tensor_copy(m1sb[:, eb, :], m1ps)

            def emit_scores(b):
                """scores = M1^T (WkA)^T + rank-1 bias terms."""
                s = st[b]
                m1sb, wk_p = s["m1sb"], s["wk_p"]
                urow, w2row = s["urow"], s["w2row"]
                bq_row, bk_row = s["bq_row"], s["bk_row"]
                scores = [
                    psp.tile([P, C], F32, tag="scores", bufs=4, name=f"sc{b}_{cb}")
                    for cb in range(CB)
                ]
                s["scores"] = scores
                for cb in range(CB):
                    for eb in range(CB):
                        nc.tensor.matmul(
                            scores[cb], m1sb[:, eb, ts(cb, P)], wk_p[:, eb, :],
                            start=(eb == 0), stop=False,
                        )
                    nc.tensor.matmul(
                        scores[cb], urow[0:1, ts(cb, P)], bk_row,
                        start=False, stop=False,
                    )
                    nc.tensor.matmul(
                        scores[cb], bq_row[0:1, ts(cb, P)], w2row,
                        start=False, stop=True,
                    )

            def emit_softmax(b):
                s = st[b]
                scores = s["scores"]
                e_sb = sbp.tile([P, CB, C], BF16, tag="e", bufs=1, name=f"e{b}")
                rinv = sbp.tile([P, CB], F32, tag="rinv", bufs=1, name=f"ri{b}")
                s["e"], s["rinv"] = e_sb, rinv
                for cb in range(CB):
                    rs = sbp.tile([P, 1], F32, tag="rs", bufs=2, name=f"rs{b}{cb}")
                    nc.scalar.activation(
                        e_sb[:, cb, :], scores[cb], AF.Exp,
                        bias=0.0, scale=SCALE, accum_out=rs,
                    )
                    nc.vector.reciprocal(rinv[:, cb : cb + 1], rs)

            def emit_wor_ft(b):
                """WoR = rinv . Wo^T (bf16); Ft = e^T WoR [d, o];
                fbo = Ft^T bv' (per-o bias from folded v bias)."""
                s = st[b]
                e_sb, rinv, bvbh = s["e"], s["rinv"], s["bvbh"]
                wor = sbp.tile([P, CB, C], BF16, tag="wor", bufs=1, name=f"wo{b}")
                for cb in range(CB):
                    nc.vector.tensor_scalar_mul(
                        wor[:, cb, :], wot[:, cb, :], rinv[:, cb : cb + 1]
                    )
                ftsb = sbp.tile([P, CB, C], FP16, tag="ftsb", bufs=1, name=f"ft{b}")
                s["ftsb"] = ftsb
                for db in range(CB):
                    ftps = psp.tile([P, C], F32, tag="work", bufs=4,
                                    name=f"ftp{b}_{db}")
                    for cb in range(CB):
                        nc.tensor.matmul(
                            ftps, e_sb[:, cb, ts(db, P)], wor[:, cb, :],
                            start=(cb == 0), stop=(cb == CB - 1),
                        )
                    nc.vector.tensor_copy(ftsb[:, db, :], ftps)
                # fbo[o] = sum_d Ft[d,o] bv'[d]; fold into epilogue bias
                bobf = sbp.tile([P, CB], F32, tag="bobf", bufs=2, name=f"bo{b}")
                s["bobf"] = bobf
                fbo = sbp.tile([P, CB], F32, tag="fbo", bufs=2, name=f"fb{b}")
                for ob in range(CB):
                    fbps = psp.tile([P, 1], F32, tag="work", bufs=4,
                                    name=f"fbp{b}_{ob}")
                    for db in range(CB):
                        nc.tensor.matmul(
                            fbps, ftsb[:, db, ts(ob, P)], bvbh[:, db : db + 1],
                            start=(db == 0), stop=(db == CB - 1),
                        )
                    nc.vector.tensor_copy(fbo[:, ob : ob + 1], fbps)
                nc.vector.tensor_add(bobf, fbo, bob)

            def emit_fwt(b):
                """FWt = A . (Wv^T Ft)  [e, o] — A folded into the evac so the
                epilogue can consume raw bf16 x."""
                s = st[b]
                ftsb, A_ = s["ftsb"], s["A_"]
                fwsb = sbp.tile([P, CB, C], FP16, tag="fwsb", bufs=1, name=f"fw{b}")
                s["fwsb"] = fwsb
                for eb in range(CB):
                    fwps = psp.tile([P, C], F32, tag="work", bufs=4,
                                    name=f"fwp{b}_{eb}")
                    for db in range(CB):
                        nc.tensor.matmul(
                            fwps, wvnt[:, db, ts(eb, P)], ftsb[:, db, :],
                            start=(db == 0), stop=(db == CB - 1),
                        )
                    nc.vector.tensor_scalar_mul(
                        fwsb[:, eb, :], fwps, A_[:, eb : eb + 1]
                    )

            def emit_ef(b):
                """out = (A.FWt)^T x + (bobf)1^T;  y = x + out."""
                s = st[b]
                fwsb, xsb, bobf = s["fwsb"], s["xsb"], s["bobf"]
                for nsl in range(NS):
                    for ob in range(CB):
                        pf = psp.tile([P, NSL], F32, tag="work", bufs=4,
                                      name=f"pf{b}{nsl}{ob}")
                        for eb in range(CB):
                            nc.tensor.matmul(
                                pf, fwsb[:, eb, ts(ob, P)],
                                xsb[:, eb, ts(nsl, NSL)],
                                start=(eb == 0), stop=(eb == CB - 1),
                            )
                        yt = sbp.tile([P, NSL], F32, tag="yt", bufs=3,
                                      name=f"yt{b}{nsl}{ob}")
                        nc.vector.scalar_tensor_tensor(
                            yt, pf, bobf[:, ob : ob + 1],
                            xsb[:, ob, ts(nsl, NSL)],
                            op0=OP.add, op1=OP.add,
                        )
                        nc.sync.dma_start(yview[b][:, ob, ts(nsl, NSL)], yt)

            # ---- prologue ----
            emit_load(0)
            # HAM warm-up: keep TensorE busy through the prologue so the Gram
            # phase starts at full clock. The dummy accumulator drains to DRAM
            # so the chain is not dead code.
            zsb = sg.tile([P, NSL], BF16, name="zsb")
            nc.gpsimd.memset(zsb, 0.0)
            pdum = psp.tile([P, NSL], F32, tag="work", bufs=4, name="pdum")
            for i in range(24):
                nc.tensor.matmul(
                    pdum, zsb[:, :P], zsb, start=(i == 0), stop=False
                )
            for cb in range(CB):
                nc.tensor.matmul(
                    pdum, st[0]["xsb"][:, cb, ts(0, P)], zsb,
                    start=False, stop=(cb == CB - 1),
                )
            dsb = sg.tile([1, 1], F32, name="dsb")
            nc.vector.tensor_copy(dsb, pdum[0:1, 0:1])
            dscr = drp.tile([1], F32, name="dscr")
            nc.sync.dma_start(dscr.rearrange("(a c) -> a c", a=1), dsb)
            # ---- constants, loaded once ----
            gfwd = sg.tile([P, CB, GROUPS], F32)
            nc.sync.dma_start(gfwd, gfwd_d[:])
            gbwd = sg.tile([GROUPS, CB, P], F32)
            nc.sync.dma_start(gbwd, gbwd_d[:])
            wall = sg.tile([P, 3, CB, C], FP16)
            for wi, w in enumerate(WKEYS):
                for cb in range(CB):
                    nc.sync.dma_start(wall[:, wi, cb, :], wt_dram[w][ts(cb, P), :])
            wvnt = sg.tile([P, CB, C], FP16)
            nc.sync.dma_start(wvnt, wvnt_d[:].rearrange("(cb p) e -> p cb e", p=P))
            wot = sg.tile([P, CB, C], FP16)
            nc.sync.dma_start(wot, wot_d[:].rearrange("(cb p) o -> p cb o", p=P))
            gam = sg.tile([P, CB], F32)
            nc.sync.dma_start(gam, gamma_d[:].rearrange("(cb p) -> p cb", p=P))
            bet = sg.tile([P, CB], F32)
            nc.sync.dma_start(bet, beta_d[:].rearrange("(cb p) -> p cb", p=P))
            bob = sg.tile([P, CB], F32)
            nc.sync.dma_start(bob, bo_d[:].rearrange("(cb p) -> p cb", p=P))
            bqv = sg.tile([1, C], F32)
            nc.sync.dma_start(bqv, bq_d[:].rearrange("(a c) -> a c", a=1))
            bkv = sg.tile([1, C], F32)
            nc.sync.dma_start(bkv, bk_d[:].rearrange("(a c) -> a c", a=1))
            bvv = sg.tile([1, C], F32)
            nc.sync.dma_start(bvv, bv_d[:].rearrange("(a c) -> a c", a=1))
            eps_g = sg.tile([GROUPS, 1], F32)
            nc.vector.memset(eps_g, EPS)
            bias_dram["q"], bias_dram["k"], bias_dram["v"] = bqv, bkv, bvv

            emit_stats(0)
            emit_a2(0)
            # ---- software-pipelined emission across the two batches ----
            emit_xx(0)
            emit_m1(0)
            emit_scores(0)
            emit_softmax(0)
            emit_load(1)
            emit_xx(1)          # fills PE during batch-0 softmax
            emit_wor_ft(0)
            emit_fwt(0)
            emit_stats(1)
            emit_ef(0)
            emit_a2(1)
            emit_m1(1)
            emit_scores(1)
            emit_softmax(1)
            emit_wor_ft(1)
            emit_fwt(1)
            emit_ef(1)

    nc.finalize()
    return nc


def _get_nc():
    if "nc" not in _NC_CACHE:
        _NC_CACHE["nc"] = _build_nc()
    return _NC_CACHE["nc"]


def _make_consts():
    gfwd = np.zeros((P, CB, GROUPS), np.float32)
    gbwd = np.zeros((GROUPS, CB, P), np.float32)
    for cb in range(CB):
        for p in range(P):
            g = (cb * P + p) // 16
            gfwd[p, cb, g] = 1.0 / 16.0
            gbwd[g, cb, p] = 1.0
    return gfwd, gbwd


def kernel(x, gamma, beta, Wq, bq, Wk, bk, Wv, bv, Wo, bo):
    global LAST_RESULT
    from concourse.bass_utils import run_bass_kernel_spmd

    import ml_dtypes

    BF = ml_dtypes.bfloat16
    H = np.float16
    x = np.ascontiguousarray(np.asarray(x, np.float32)).reshape(16, C, N)
    xbf = np.ascontiguousarray(x.astype(BF))
    xth = np.ascontiguousarray(np.transpose(x, (0, 2, 1)).astype(H))
    gfwd, gbwd = _make_consts()
    shared = {
        "wqt": np.ascontiguousarray(np.asarray(Wq, np.float32).T.astype(H)),
        "wkt": np.ascontiguousarray(np.asarray(Wk, np.float32).T.astype(H)),
        "wvt": np.ascontiguousarray(np.asarray(Wv, np.float32).T.astype(H)),
        "wvnt": np.ascontiguousarray(np.asarray(Wv, np.float32).astype(H)),
        "wot": np.ascontiguousarray(np.asarray(Wo, np.float32).T.astype(H)),
        "bq": np.ascontiguousarray(np.asarray(bq, np.float32)),
        "bk": np.ascontiguousarray(np.asarray(bk, np.float32)),
        "bv": np.ascontiguousarray(np.asarray(bv, np.float32)),
        "bo": np.ascontiguousarray(np.asarray(bo, np.float32)),
        "gamma": np.ascontiguousarray(np.asarray(gamma, np.float32)),
        "beta": np.ascontiguousarray(np.asarray(beta, np.float32)),
        "gfwd": gfwd,
        "gbwd": gbwd,
    }
    in_maps = [
        dict(
            shared,
            xs=np.ascontiguousarray(xbf[BB * i : BB * (i + 1)]),
            xt=np.ascontiguousarray(xth[BB * i : BB * (i + 1)]),
        )
        for i in range(8)
    ]
    nc = _get_nc()
    import os

    trace = os.environ.get("KERNEL_TRACE") == "1"
    res = run_bass_kernel_spmd(nc, in_maps, core_ids=list(range(8)), trace=trace)
    LAST_RESULT = res
    y = np.concatenate([r["y"] for r in res.results], axis=0)
    return y.reshape(16, C, 64, 64)
